# revision 9
# baseline (speedup 1.0000x reference)
"""DIoU regression loss on 8 Trainium2 NeuronCores (data-parallel).

loss = sum(1 - clip(diou(pred_i, gt_i), -1, 1)) / (N + 1e-4) over N=4M boxes.

Sharding: each core gets a contiguous slab of R = 128*T*K rows; the last
core's slab is padded with identical unit boxes whose diou == 1, so padded
rows contribute 0 to sum(1 - diou). Each core returns per-partition sums of
clip(diou); the host combines: loss = (N_padded - sum(diou)) / (N + 1e-4).

Math (equivalent to the det3d corner-based reference):
  full extents per box: Ex = w*cos(r) + l*sin(r), Ey = l*cos(r) - w*sin(r),
  Ez = h.  For a dim with centers (cp, cg) and full extents (Ep, Eg):
    inter = relu(min(Ep, Eg, (Ep+Eg)/2 - |cg-cp|))
    outer = relu(max(Ep, Eg, (Ep+Eg)/2 + |cg-cp|))
  (identical to min/max over the reference's corner0/corner2 expressions).
"""

import numpy as np

import concourse.bacc as bacc
import concourse.mybir as mybir
import concourse.tile as tile
from concourse import bass_utils

P = 128          # SBUF partitions
T = 652          # rows per partition per tile
K = 6            # tiles per core
NCORES = 8
RCORE = P * T * K            # 500,736 rows per core
NPAD = RCORE * NCORES        # 4,005,888
NREAL = 4_000_000
D = 9
F32 = mybir.dt.float32
CT = mybir.dt.float32        # compute dtype for elementwise temps
HALF_PI = float(np.pi / 2)

AF = mybir.ActivationFunctionType
OP = mybir.AluOpType

_PAD_ROW = np.array([0, 0, 0, 1, 1, 1, 0, 0, 0], dtype=np.float32)

_CACHE = {}
_TRACE = False
_LAST = None
_DEBUG = False


def _build():
    nc = bacc.Bacc("TRN2", target_bir_lowering=False, debug=False,
                   num_devices=NCORES)
    pred = nc.dram_tensor("pred", [RCORE, D], F32, kind="ExternalInput").ap()
    gt = nc.dram_tensor("gt", [RCORE, D], F32, kind="ExternalInput").ap()
    out = nc.dram_tensor("out", [P, 1], F32, kind="ExternalOutput").ap()
    dbg = {}
    if _DEBUG:
        for nm in ("cs_p", "Ex_p", "diou", "acc"):
            shp = [P, K] if nm == "acc" else [P, T]
            dbg[nm] = nc.dram_tensor(f"dbg_{nm}", shp, F32,
                                     kind="ExternalOutput").ap()

    predv = pred.rearrange("(k p t) d -> k p t d", p=P, t=T)
    gtv = gt.rearrange("(k p t) d -> k p t d", p=P, t=T)

    with tile.TileContext(nc) as tc:
        with (
            tc.tile_pool(name="raw", bufs=2) as raw,
            tc.tile_pool(name="tmp", bufs=1) as tmp,
            tc.tile_pool(name="one", bufs=1) as one,
        ):
            acc = one.tile([P, K], F32, tag="acc", name="acc")
            halfpi = one.tile([P, 1], F32, tag="halfpi", name="halfpi")
            nc.vector.memset(halfpi, HALF_PI)

            for i in range(K):
                praw = raw.tile([P, T, D], F32, tag="praw", name="praw")
                graw = raw.tile([P, T, D], F32, tag="graw", name="graw")
                nc.sync.dma_start(out=praw, in_=predv[i])
                nc.sync.dma_start(out=graw, in_=gtv[i])

                def t(tag):
                    return tmp.tile([P, T], CT, tag=tag, name=tag)

                # --- per-box: extents Ex, Ey and volume ---
                ext = {}   # (box, dim) -> extent tile;  vols[box]
                vols = {}
                for box, rw in (("p", praw), ("g", graw)):
                    x_, y_, z_, w_, l_, h_, r_ = (rw[:, :, c] for c in range(7))
                    sn = t(f"sn_{box}")
                    cs = t(f"cs_{box}")
                    nc.scalar.activation(out=sn, in_=r_, func=AF.Sin)
                    # cos(r) = sin(pi/2 - r); keeps the arg in (0.57, 1.57]
                    # (the ACT Sin spline's domain does not cover r + pi/2).
                    nc.scalar.activation(out=cs, in_=r_, func=AF.Sin,
                                         bias=halfpi, scale=-1.0)
                    t1 = t(f"t1_{box}")   # becomes Ex
                    t4 = t(f"t4_{box}")   # becomes Ey
                    t2 = t("t2")
                    t3 = t("t3")
                    nc.vector.tensor_mul(t1, w_, cs)
                    nc.vector.tensor_mul(t2, l_, sn)
                    nc.vector.tensor_mul(t3, w_, sn)
                    nc.vector.tensor_mul(t4, l_, cs)
                    nc.vector.tensor_add(t1, t1, t2)      # Ex
                    nc.vector.tensor_sub(t4, t4, t3)      # Ey
                    vol = t(f"vol_{box}")
                    nc.gpsimd.tensor_tensor(out=vol, in0=w_, in1=l_, op=OP.mult)
                    nc.gpsimd.tensor_tensor(out=vol, in0=vol, in1=h_, op=OP.mult)
                    ext[(box, 0)] = (x_, t1)
                    ext[(box, 1)] = (y_, t4)
                    ext[(box, 2)] = (z_, h_)
                    vols[box] = vol

                inters = []
                outers2 = []
                deltas2 = []
                for dim in range(3):
                    cp, Ep = ext[("p", dim)]
                    cg, Eg = ext[("g", dim)]
                    delta = t(f"delta_{dim}")    # later squared in place
                    nc.vector.tensor_sub(delta, cg, cp)
                    ad = t("ad")
                    nc.scalar.activation(out=ad, in_=delta, func=AF.Abs)
                    m = t("m")
                    M = t("M")
                    S = t("S")
                    nc.vector.tensor_tensor(out=m, in0=Ep, in1=Eg, op=OP.min)
                    nc.vector.tensor_tensor(out=M, in0=Ep, in1=Eg, op=OP.max)
                    nc.vector.tensor_add(S, Ep, Eg)
                    t1d = t("t1d")
                    t2d = t("t2d")
                    # (S * 0.5) -/+ ad
                    nc.vector.scalar_tensor_tensor(out=t1d, in0=S, scalar=0.5,
                                                   in1=ad, op0=OP.mult,
                                                   op1=OP.subtract)
                    nc.vector.scalar_tensor_tensor(out=t2d, in0=S, scalar=0.5,
                                                   in1=ad, op0=OP.mult,
                                                   op1=OP.add)
                    i0 = t(f"i_{dim}")
                    nc.vector.tensor_tensor(out=i0, in0=m, in1=t1d, op=OP.min)
                    nc.vector.tensor_scalar_max(i0, i0, 0.0)   # inter_d
                    o0 = t("o0")
                    nc.vector.tensor_tensor(out=o0, in0=M, in1=t2d, op=OP.max)
                    nc.vector.tensor_scalar_max(o0, o0, 0.0)
                    o2 = t(f"o2_{dim}")
                    nc.scalar.activation(out=o2, in_=o0, func=AF.Square)
                    nc.scalar.activation(out=delta, in_=delta, func=AF.Square)
                    inters.append(i0)
                    outers2.append(o2)
                    deltas2.append(delta)

                # idiag = dx2+dy2+dz2 (into deltas2[0]); odiag into outers2[0]
                idiag = deltas2[0]
                nc.gpsimd.tensor_tensor(out=idiag, in0=idiag, in1=deltas2[1], op=OP.add)
                nc.gpsimd.tensor_tensor(out=idiag, in0=idiag, in1=deltas2[2], op=OP.add)
                odiag = outers2[0]
                nc.gpsimd.tensor_tensor(out=odiag, in0=odiag, in1=outers2[1], op=OP.add)
                nc.gpsimd.tensor_tensor(out=odiag, in0=odiag, in1=outers2[2], op=OP.add)
                iv = inters[0]
                nc.vector.tensor_mul(iv, iv, inters[1])
                nc.vector.tensor_mul(iv, iv, inters[2])
                un = vols["p"]
                nc.vector.tensor_add(un, un, vols["g"])
                nc.vector.tensor_sub(un, un, iv)
                nc.vector.reciprocal_approx_fast(out=un, in_=un)        # 1/union
                nc.vector.reciprocal_approx_fast(out=odiag, in_=odiag)  # 1/odiag
                nc.vector.tensor_mul(iv, iv, un)          # r1
                nc.vector.tensor_mul(idiag, idiag, odiag)  # r2
                nc.vector.tensor_sub(iv, iv, idiag)       # diou (uncl.)
                # clip to [-1,1] and row-sum into acc[:, i]
                nc.vector.tensor_scalar(out=iv, in0=iv, scalar1=1.0,
                                        scalar2=-1.0, op0=OP.min, op1=OP.max)
                nc.vector.tensor_reduce(acc[:, i:i + 1], iv,
                                        axis=mybir.AxisListType.X, op=OP.add)
                if _DEBUG and i == 0:
                    nc.sync.dma_start(out=dbg["diou"], in_=iv)
                    nc.sync.dma_start(out=dbg["cs_p"], in_=ext[("p", 0)][1])

            if _DEBUG:
                nc.sync.dma_start(out=dbg["acc"], in_=acc)

            red = one.tile([P, 1], F32, tag="red", name="red")
            nc.vector.tensor_reduce(red, acc, axis=mybir.AxisListType.X,
                                    op=OP.add)
            nc.sync.dma_start(out=out, in_=red)

    nc.compile()
    return nc


def kernel(box_pred, box_gt):
    global _LAST
    box_pred = np.asarray(box_pred, dtype=np.float32)
    box_gt = np.asarray(box_gt, dtype=np.float32)
    n = box_pred.shape[0]
    assert n == NREAL, f"kernel hardcoded for N={NREAL}, got {n}"

    if "nc" not in _CACHE:
        _CACHE["nc"] = _build()
    nc = _CACHE["nc"]

    npad = NPAD - NREAL
    pad = np.broadcast_to(_PAD_ROW, (npad, D))
    in_maps = []
    for c in range(NCORES):
        lo, hi = c * RCORE, (c + 1) * RCORE
        if hi <= NREAL:
            p_sl, g_sl = box_pred[lo:hi], box_gt[lo:hi]
        else:
            p_sl = np.concatenate([box_pred[lo:NREAL], pad], axis=0)
            g_sl = np.concatenate([box_gt[lo:NREAL], pad], axis=0)
        in_maps.append({"pred": p_sl, "gt": g_sl})

    kw = dict(trace=True, trace_cores=[0]) if _TRACE else {}
    res = bass_utils.run_bass_kernel_spmd(nc, in_maps,
                                          core_ids=list(range(NCORES)), **kw)
    _LAST = res
    total_diou = sum(
        float(res.results[c]["out"].astype(np.float64).sum())
        for c in range(NCORES)
    )
    loss = (NPAD - total_diou) / (NREAL + 1e-4)
    return np.float32(loss)
